# revision 22
# baseline (speedup 1.0000x reference)
"""Trainium2 Bass kernel for the GNN attention module
(scatter-mean -> dense+tanh -> attention coefs -> weighted scatter-add),
data-parallel over graphs on 8 NeuronCores.

Self-contained: hardcodes N=2000000, D=64, G=8192, 8 cores.

Per core (contiguous node/graph shard, local graph ids):
  pass 1: per 128-node block, one-hot(graph) matmul against [x | 1] gives
          transposed seg-sum + counts for a 32-wide sliding graph window in
          PSUM; windows are accumulated into an SBUF accumulator [65, GCP]
          at a register-dynamic column offset.
  mid:    inv = 1/max(counts,1) broadcast via K=1 matmul; meanT = segT*inv;
          tgT = tanh(W^T @ meanT)  (bf16, resident in SBUF [64, GCP])
  pass 2: per block, dots = xT_bf16^T @ tgT[:, window] on PE; pick via
          one-hot mult + reduce; sigmoid -> coefs; coefs folded into the
          one-hot; weighted seg-sum matmul; accumulate like pass 1.
  end:    PE-transpose the [64, GCP] accumulator back to [GCP, 64], DMA out.
"""
import os
import numpy as np
from contextlib import ExitStack

import ml_dtypes

P = 128          # partitions / nodes per block
T = 32           # blocks per mega-tile
NT = P * T       # nodes per mega-tile (2048)
WIN = 32         # mega window width (graphs)
D = 64
DP1 = D + 1      # x columns + ones column
DP2 = D + 2      # + packed per-block graph offset (b32)
N_FULL = 2_000_000
G_FULL = 8192
CORES = 8
GCP = 1152       # padded local graph count (9 * 128)
NCHUNK = GCP // P

LAST_EXEC_NS = None


# ----------------------------------------------------------------------------
# host-side preprocessing
# ----------------------------------------------------------------------------

def _shard_plan(batch, size, cores):
    counts = np.bincount(batch.astype(np.int64), minlength=size)
    cum = np.concatenate([[0], np.cumsum(counts)])
    n = batch.shape[0]
    gsplit = [0]
    for k in range(1, cores):
        g = int(np.searchsorted(cum, k * n / cores))
        g = max(gsplit[-1] + 1, min(g, size - (cores - k)))
        gsplit.append(g)
    gsplit.append(size)
    nsplit = [int(cum[g]) for g in gsplit]
    return gsplit, nsplit


def _prep_core(x, batch, g0, g1, n0, n1, n_meg):
    nn = n1 - n0
    npad = n_meg * NT
    lg = (batch[n0:n1] - g0).astype(np.int64)
    gc = g1 - g0
    ghost = gc                           # pad nodes get this local graph id
    lg_full = np.full(npad, ghost, dtype=np.int64)
    lg_full[:nn] = lg

    xs_pad = np.zeros((npad, D), dtype=np.float32)
    xs_pad[:nn] = x[n0:n1]

    lgt = lg_full.reshape(n_meg, T, P)            # [t, j, p]
    c0 = np.minimum(lgt[:, 0, 0], GCP - WIN)      # mega window base
    b32 = lgt - c0[:, None, None]
    assert b32.min() >= 0 and b32.max() < WIN, (b32.min(), b32.max())
    assert ghost + 1 <= GCP

    b32 = b32.transpose(0, 2, 1).astype(np.float32)   # [t, p, j]

    xs4 = np.ones((n_meg, P, T, DP2), dtype=np.float32)
    xs4[:, :, :, :D] = xs_pad.reshape(n_meg, T, P, D).transpose(0, 2, 1, 3)
    xs4 = xs4.astype(ml_dtypes.bfloat16)
    xs4[:, :, :, D + 1] = b32.astype(ml_dtypes.bfloat16)
    xtb = xs_pad.reshape(n_meg, T, P, D).transpose(0, 1, 3, 2)  # [t, j, d, q]
    xts = np.ascontiguousarray(
        xtb.reshape(n_meg, T // 2, 2, D, P).transpose(0, 2, 3, 1, 4)
        .reshape(n_meg, P, (T // 2) * P)
    ).astype(ml_dtypes.bfloat16)

    c0s = np.zeros((1, n_meg), dtype=np.int32)
    c0s[0, :] = c0
    nm2 = n_meg // 2
    xs4 = np.ascontiguousarray(
        xs4.reshape(nm2, 2, P, T * DP2).transpose(0, 2, 1, 3)
    ).reshape(nm2, P, 2 * T, DP2)
    xts = np.ascontiguousarray(
        xts.reshape(nm2, 2, P, (T // 2) * P).transpose(0, 2, 1, 3)
    ).reshape(nm2, P, T, P)
    return {"xs": xs4, "xts": xts, "c0s": c0s}, gc


def _host_consts():
    iota = np.broadcast_to(np.arange(WIN, dtype=np.float32), (P, T, WIN)).copy()
    ident = np.eye(P, dtype=np.float32)
    return iota, ident


# ----------------------------------------------------------------------------
# device kernel
# ----------------------------------------------------------------------------

def build_nc(n_meg):
    from concourse import mybir
    import concourse.tile as tile
    import concourse.bacc as bacc

    f32 = mybir.dt.float32
    bf16 = mybir.dt.bfloat16
    i32 = mybir.dt.int32
    AF = mybir.ActivationFunctionType
    ALU = mybir.AluOpType
    ENG = mybir.EngineType

    nc = bacc.Bacc("TRN2", target_bir_lowering=False, debug=False,
                   num_devices=CORES)

    xs = nc.dram_tensor("xs", [n_meg // 2, P, 2 * T, DP2], bf16, kind="ExternalInput").ap()
    xts = nc.dram_tensor("xts", [n_meg // 2, P, T, P], bf16, kind="ExternalInput").ap()
    c0s = nc.dram_tensor("c0s", [1, n_meg], i32, kind="ExternalInput").ap()
    wmat = nc.dram_tensor("wmat", [D, D], f32, kind="ExternalInput").ap()
    iotac = nc.dram_tensor("iotac", [P, T, WIN], f32, kind="ExternalInput").ap()
    identc = nc.dram_tensor("identc", [P, P], f32, kind="ExternalInput").ap()
    out = nc.dram_tensor("out", [GCP, D], f32, kind="ExternalOutput").ap()
    tgscratch = nc.dram_tensor("tgscratch", [D, GCP], bf16, kind="Internal").ap()

    with tile.TileContext(nc) as tc, ExitStack() as ctx:
        cpool = ctx.enter_context(tc.tile_pool(name="const", bufs=1))
        px = ctx.enter_context(tc.tile_pool(name="px", bufs=4))
        pxt = ctx.enter_context(tc.tile_pool(name="pxt", bufs=4))
        pm = ctx.enter_context(tc.tile_pool(name="pm", bufs=3))
        pk = ctx.enter_context(tc.tile_pool(name="pk", bufs=4))
        pp = ctx.enter_context(tc.tile_pool(name="pp", bufs=1, space="PSUM"))
        ppd = ctx.enter_context(tc.tile_pool(name="ppd", bufs=2, space="PSUM"))
        pp1 = ctx.enter_context(tc.tile_pool(name="pp1", bufs=3, space="PSUM"))

        iota_sb = cpool.tile([P, T, WIN], f32)
        nc.sync.dma_start(iota_sb[:], iotac[:])
        ident_sb = cpool.tile([P, P], f32)
        nc.sync.dma_start(ident_sb[:], identc[:])
        w_sb = cpool.tile([D, D], f32)
        nc.sync.dma_start(w_sb[:], wmat[:])
        c0_sb = cpool.tile([1, n_meg], i32)
        nc.sync.dma_start(c0_sb[:], c0s[:])
        ones1 = cpool.tile([1, D], f32)
        nc.gpsimd.memset(ones1[:], 1.0)

        acc1 = cpool.tile([DP1, GCP], f32)
        nc.vector.memset(acc1[:], 0.0)
        acc2 = cpool.tile([DP1, GCP], f32)
        nc.vector.memset(acc2[:], 0.0)
        tgT = cpool.tile([P, 2, GCP], bf16)
        nc.vector.memset(tgT[:], 0.0)

        import concourse.bass as bass

        def c0_of(t, engines):
            return nc.values_load(
                c0_sb[0:1, t:t + 1], engines=engines,
                min_val=0, max_val=GCP - WIN, skip_runtime_bounds_check=True)

        def build_M(b32_ap, eng=None):
            m = pm.tile([P, T, WIN], bf16, tag="M")
            (eng or nc.vector).tensor_tensor(
                out=m[:], in0=iota_sb[:],
                in1=b32_ap.to_broadcast([P, T, WIN]),
                op=ALU.is_equal)
            return m

        # ---------------- pass 1: transposed seg-sum + counts --------------
        for tt in range(n_meg // 2):
            xs_t = px.tile([P, 2 * T, DP2], bf16, tag="xs")
            nc.sync.dma_start(xs_t[:], xs[tt])
            for h in range(2):
                t = 2 * tt + h
                m = build_M(xs_t[:, h * T:(h + 1) * T, D + 1])
                ps1 = pp1.tile([DP1, WIN], f32, tag="pacc")
                for j in range(T):
                    nc.tensor.matmul(ps1[:], lhsT=xs_t[:, h * T + j, 0:DP1],
                                     rhs=m[:, j, :],
                                     start=(j == 0), stop=(j == T - 1))
                c0v = c0_of(t, engines=[ENG.DVE])
                a = acc1[:, bass.ds(c0v, WIN)]
                nc.vector.tensor_tensor(out=a, in0=a, in1=ps1[:], op=ALU.add)

        # ---------------- mid: tgT = tanh(W^T @ (segT * inv)) ---------------
        cnt = cpool.tile([1, GCP], f32)
        nc.sync.dma_start(cnt[:], acc1[D:DP1, :])   # move counts row to part 0
        nc.vector.tensor_scalar_max(cnt[:], cnt[:], 1.0)
        inv = cpool.tile([1, GCP], f32)
        nc.vector.reciprocal(inv[:], cnt[:])
        meanT = cpool.tile([D, GCP], f32)
        CH = 512
        nchunks = (GCP + CH - 1) // CH
        for c in range(nchunks):
            w = min(CH, GCP - c * CH)
            sl = slice(c * CH, c * CH + w)
            psb = pp.tile([D, CH], f32, tag="mid")
            nc.tensor.matmul(psb[:, :w], lhsT=ones1[:], rhs=inv[:, sl],
                             start=True, stop=True)
            nc.vector.tensor_tensor(out=meanT[:, sl], in0=acc1[0:D, sl],
                                    in1=psb[:, :w], op=ALU.mult)
        for c in range(nchunks):
            w = min(CH, GCP - c * CH)
            sl = slice(c * CH, c * CH + w)
            psg = pp.tile([D, CH], f32, tag="mid")
            nc.tensor.matmul(psg[:, :w], lhsT=w_sb[:], rhs=meanT[:, sl],
                             start=True, stop=True)
            nc.scalar.activation(tgT[0:D, 0, sl], psg[:, :w], AF.Tanh)

        nc.sync.dma_start(tgscratch[:], tgT[0:D, 0, :])
        nc.sync.dma_start(tgT[D:P, 1, :], tgscratch[:])

        # ---------------- pass 2: coefs + weighted seg-sum ------------------
        for tt in range(n_meg // 2):
            xs_t = px.tile([P, 2 * T, DP2], bf16, tag="xs")
            nc.sync.dma_start(xs_t[:], xs[tt])
            xts_t = pxt.tile([P, T, P], bf16, tag="xts")
            nc.sync.dma_start(xts_t[:], xts[tt])
            for h in range(2):
                t = 2 * tt + h
                c0a = c0_of(t, engines=[ENG.Activation])
                tgwin = pk.tile([P, 2, WIN], bf16, tag="tgwin")
                nc.scalar.copy(tgwin[:], tgT[:, :, bass.ds(c0a, WIN)])

                m = build_M(xs_t[:, h * T:(h + 1) * T, D + 1])
                psD = ppd.tile([P, T, WIN], f32, tag="psD")
                for jj in range(T // 2):
                    nc.tensor.matmul(psD[:, 2 * jj:2 * jj + 2, :],
                                     lhsT=xts_t[:, h * (T // 2) + jj, :],
                                     rhs=tgwin[:, :, :],
                                     start=True, stop=True)
                dsb = pm.tile([P, T, WIN], bf16, tag="dsb")
                nc.scalar.copy(dsb[:], psD[:])
                a_pick = pm.tile([P, T, WIN], bf16, tag="apick")
                nc.vector.tensor_tensor(out=a_pick[:], in0=m[:], in1=dsb[:],
                                        op=ALU.mult)
                s = pk.tile([P, T], f32, tag="s")
                nc.vector.tensor_reduce(
                    out=s[:], in_=a_pick[:], axis=mybir.AxisListType.X,
                    op=ALU.add)
                coef = pk.tile([P, T], bf16, tag="coef")
                nc.scalar.activation(coef[:], s[:], AF.Sigmoid)
                cexp = pm.tile([P, T, WIN], bf16, tag="cexp")
                nc.scalar.copy(cexp[:], coef[:].to_broadcast([P, T, WIN]))
                mp = pm.tile([P, T, WIN], bf16, tag="Mp")
                nc.vector.tensor_tensor(out=mp[:], in0=m[:], in1=cexp[:],
                                        op=ALU.mult)
                ps3 = pp1.tile([DP1, WIN], f32, tag="pacc")
                for j in range(T):
                    nc.tensor.matmul(ps3[:], lhsT=xs_t[:, h * T + j, 0:DP1],
                                     rhs=mp[:, j, :],
                                     start=(j == 0), stop=(j == T - 1))
                c0v = c0_of(t, engines=[ENG.DVE])
                a = acc2[:, bass.ds(c0v, WIN)]
                nc.vector.tensor_tensor(out=a, in0=a, in1=ps3[:], op=ALU.add)

        # ---------------- end: transpose acc2 -> out ------------------------
        for c in range(NCHUNK):
            pst = pp.tile([P, D], f32, tag="mid")
            nc.tensor.transpose(pst[:], acc2[0:D, c * P:(c + 1) * P],
                                ident_sb[0:D, 0:D])
            oc = pk.tile([P, D], f32, tag="oc")
            nc.scalar.copy(oc[:], pst[:])
            nc.sync.dma_start(out[c * P:(c + 1) * P, :], oc[:])

    nc.compile()
    return nc


# ----------------------------------------------------------------------------
# entry point
# ----------------------------------------------------------------------------

_CACHE = {}


def kernel(x, batch, size, W):
    global LAST_EXEC_NS
    from concourse import bass_utils

    x = np.asarray(x, dtype=np.float32)
    batch_np = np.asarray(batch).astype(np.int64)
    W = np.asarray(W, dtype=np.float32)
    n = x.shape[0]
    size = int(size)
    cores = CORES

    gsplit, nsplit = _shard_plan(batch_np, size, cores)
    max_nodes = max(nsplit[k + 1] - nsplit[k] for k in range(cores))
    n_meg = max(1, -(-max_nodes // NT))
    n_meg += n_meg % 2   # even, for paired loads

    iota, ident = _host_consts()
    in_maps = []
    gcs = []
    for k in range(cores):
        m, gc = _prep_core(x, batch_np, gsplit[k], gsplit[k + 1],
                           nsplit[k], nsplit[k + 1], n_meg)
        m["wmat"] = W
        m["iotac"] = iota
        m["identc"] = ident
        in_maps.append(m)
        gcs.append(gc)

    if n_meg not in _CACHE:
        _CACHE[n_meg] = build_nc(n_meg)
    nc = _CACHE[n_meg]

    trace = os.environ.get("BASS_KERNEL_TRACE", "0") == "1"
    res = bass_utils.run_bass_kernel_spmd(nc, in_maps,
                                          core_ids=list(range(cores)),
                                          trace=trace)
    LAST_EXEC_NS = res.exec_time_ns
    outs = [res.results[k]["out"][:gcs[k]] for k in range(cores)]
    full = np.concatenate(outs, axis=0)
    if full.shape[0] < size:
        full = np.concatenate(
            [full, np.zeros((size - full.shape[0], D), np.float32)], axis=0)
    return np.ascontiguousarray(full[:size], dtype=np.float32)


# revision 23
# speedup vs baseline: 1.1114x; 1.1114x over previous
"""Trainium2 Bass kernel for the GNN attention module
(scatter-mean -> dense+tanh -> attention coefs -> weighted scatter-add),
data-parallel over graphs on 8 NeuronCores.

Self-contained: hardcodes N=2000000, D=64, G=8192, 8 cores.

Per core (contiguous node/graph shard, local graph ids):
  pass 1: per 128-node block, one-hot(graph) matmul against [x | 1] gives
          transposed seg-sum + counts for a 32-wide sliding graph window in
          PSUM; windows are accumulated into an SBUF accumulator [65, GCP]
          at a register-dynamic column offset.
  mid:    inv = 1/max(counts,1) broadcast via K=1 matmul; meanT = segT*inv;
          tgT = tanh(W^T @ meanT)  (bf16, resident in SBUF [64, GCP])
  pass 2: per block, dots = xT_bf16^T @ tgT[:, window] on PE; pick via
          one-hot mult + reduce; sigmoid -> coefs; coefs folded into the
          one-hot; weighted seg-sum matmul; accumulate like pass 1.
  end:    PE-transpose the [64, GCP] accumulator back to [GCP, 64], DMA out.
"""
import os
import numpy as np
from contextlib import ExitStack

import ml_dtypes

P = 128          # partitions / nodes per block
T = 32           # blocks per mega-tile
NT = P * T       # nodes per mega-tile (2048)
WIN = 32         # mega window width (graphs)
D = 64
DP1 = D + 1      # x columns + ones column
DP2 = D + 2      # + packed per-block graph offset (b32)
N_FULL = 2_000_000
G_FULL = 8192
CORES = 8
GCP = 1152       # padded local graph count (9 * 128)
NCHUNK = GCP // P

LAST_EXEC_NS = None


# ----------------------------------------------------------------------------
# host-side preprocessing
# ----------------------------------------------------------------------------

def _shard_plan(batch, size, cores):
    counts = np.bincount(batch.astype(np.int64), minlength=size)
    cum = np.concatenate([[0], np.cumsum(counts)])
    n = batch.shape[0]
    gsplit = [0]
    for k in range(1, cores):
        g = int(np.searchsorted(cum, k * n / cores))
        g = max(gsplit[-1] + 1, min(g, size - (cores - k)))
        gsplit.append(g)
    gsplit.append(size)
    nsplit = [int(cum[g]) for g in gsplit]
    return gsplit, nsplit


def _prep_core(x, batch, g0, g1, n0, n1, n_meg):
    nn = n1 - n0
    npad = n_meg * NT
    lg = (batch[n0:n1] - g0).astype(np.int64)
    gc = g1 - g0
    ghost = gc                           # pad nodes get this local graph id
    lg_full = np.full(npad, ghost, dtype=np.int64)
    lg_full[:nn] = lg

    xs_pad = np.zeros((npad, D), dtype=np.float32)
    xs_pad[:nn] = x[n0:n1]

    lgt = lg_full.reshape(n_meg, T, P)            # [t, j, p]
    c0 = np.minimum(lgt[:, 0, 0], GCP - WIN)      # mega window base
    b32 = lgt - c0[:, None, None]
    assert b32.min() >= 0 and b32.max() < WIN, (b32.min(), b32.max())
    assert ghost + 1 <= GCP

    b32 = b32.transpose(0, 2, 1).astype(np.float32)   # [t, p, j]

    xs4 = np.ones((n_meg, P, T, DP2), dtype=np.float32)
    xs4[:, :, :, :D] = xs_pad.reshape(n_meg, T, P, D).transpose(0, 2, 1, 3)
    xs4 = xs4.astype(ml_dtypes.bfloat16)
    xs4[:, :, :, D + 1] = b32.astype(ml_dtypes.bfloat16)
    xtb = xs_pad.reshape(n_meg, T, P, D).transpose(0, 1, 3, 2)  # [t, j, d, q]
    xts = np.ascontiguousarray(
        xtb.reshape(n_meg, T // 2, 2, D, P).transpose(0, 2, 3, 1, 4)
        .reshape(n_meg, P, (T // 2) * P)
    ).astype(ml_dtypes.bfloat16)

    c0s = np.zeros((1, n_meg), dtype=np.int32)
    c0s[0, :] = c0
    return {"xs": xs4, "xts": xts, "c0s": c0s}, gc


def _host_consts():
    iota = np.broadcast_to(np.arange(WIN, dtype=np.float32), (P, T, WIN)).copy()
    ident = np.eye(P, dtype=np.float32)
    return iota, ident


# ----------------------------------------------------------------------------
# device kernel
# ----------------------------------------------------------------------------

def build_nc(n_meg):
    from concourse import mybir
    import concourse.tile as tile
    import concourse.bacc as bacc

    f32 = mybir.dt.float32
    bf16 = mybir.dt.bfloat16
    i32 = mybir.dt.int32
    AF = mybir.ActivationFunctionType
    ALU = mybir.AluOpType
    ENG = mybir.EngineType

    nc = bacc.Bacc("TRN2", target_bir_lowering=False, debug=False,
                   num_devices=CORES)

    xs = nc.dram_tensor("xs", [n_meg, P, T, DP2], bf16, kind="ExternalInput").ap()
    xts = nc.dram_tensor("xts", [n_meg, P, (T // 2) * P], bf16, kind="ExternalInput").ap()
    c0s = nc.dram_tensor("c0s", [1, n_meg], i32, kind="ExternalInput").ap()
    wmat = nc.dram_tensor("wmat", [D, D], f32, kind="ExternalInput").ap()
    iotac = nc.dram_tensor("iotac", [P, T, WIN], f32, kind="ExternalInput").ap()
    identc = nc.dram_tensor("identc", [P, P], f32, kind="ExternalInput").ap()
    out = nc.dram_tensor("out", [GCP, D], f32, kind="ExternalOutput").ap()
    tgscratch = nc.dram_tensor("tgscratch", [D, GCP], bf16, kind="Internal").ap()

    with tile.TileContext(nc) as tc, ExitStack() as ctx:
        cpool = ctx.enter_context(tc.tile_pool(name="const", bufs=1))
        px = ctx.enter_context(tc.tile_pool(name="px", bufs=4))
        pxt = ctx.enter_context(tc.tile_pool(name="pxt", bufs=4))
        pm = ctx.enter_context(tc.tile_pool(name="pm", bufs=3))
        pk = ctx.enter_context(tc.tile_pool(name="pk", bufs=4))
        pp = ctx.enter_context(tc.tile_pool(name="pp", bufs=1, space="PSUM"))
        ppd = ctx.enter_context(tc.tile_pool(name="ppd", bufs=2, space="PSUM"))
        pp1 = ctx.enter_context(tc.tile_pool(name="pp1", bufs=3, space="PSUM"))

        iota_sb = cpool.tile([P, T, WIN], f32)
        nc.sync.dma_start(iota_sb[:], iotac[:])
        ident_sb = cpool.tile([P, P], f32)
        nc.sync.dma_start(ident_sb[:], identc[:])
        w_sb = cpool.tile([D, D], f32)
        nc.sync.dma_start(w_sb[:], wmat[:])
        c0_sb = cpool.tile([1, n_meg], i32)
        nc.sync.dma_start(c0_sb[:], c0s[:])
        ones1 = cpool.tile([1, D], f32)
        nc.gpsimd.memset(ones1[:], 1.0)

        acc1 = cpool.tile([DP1, GCP], f32)
        nc.vector.memset(acc1[:], 0.0)
        acc2 = cpool.tile([DP1, GCP], f32)
        nc.vector.memset(acc2[:], 0.0)
        tgT = cpool.tile([P, 2, GCP], bf16)
        nc.vector.memset(tgT[:], 0.0)

        import concourse.bass as bass

        def c0_of(t, engines):
            return nc.values_load(
                c0_sb[0:1, t:t + 1], engines=engines,
                min_val=0, max_val=GCP - WIN, skip_runtime_bounds_check=True)

        def build_M(b32_ap, eng=None):
            m = pm.tile([P, T, WIN], bf16, tag="M")
            (eng or nc.vector).tensor_tensor(
                out=m[:], in0=iota_sb[:],
                in1=b32_ap.to_broadcast([P, T, WIN]),
                op=ALU.is_equal)
            return m

        # ---------------- pass 1: transposed seg-sum + counts --------------
        for t in range(n_meg):
            xs_t = px.tile([P, T, DP2], bf16, tag="xs")
            nc.sync.dma_start(xs_t[:], xs[t])
            m = build_M(xs_t[:, :, D + 1])
            ps1 = pp1.tile([DP1, WIN], f32, tag="pacc")
            for j in range(T):
                nc.tensor.matmul(ps1[:], lhsT=xs_t[:, j, 0:DP1], rhs=m[:, j, :],
                                 start=(j == 0), stop=(j == T - 1))
            c0v = c0_of(t, engines=[ENG.DVE])
            a = acc1[:, bass.ds(c0v, WIN)]
            nc.vector.tensor_tensor(out=a, in0=a, in1=ps1[:], op=ALU.add)

        # ---------------- mid: tgT = tanh(W^T @ (segT * inv)) ---------------
        cnt = cpool.tile([1, GCP], f32)
        nc.sync.dma_start(cnt[:], acc1[D:DP1, :])   # move counts row to part 0
        nc.vector.tensor_scalar_max(cnt[:], cnt[:], 1.0)
        inv = cpool.tile([1, GCP], f32)
        nc.vector.reciprocal(inv[:], cnt[:])
        meanT = cpool.tile([D, GCP], f32)
        CH = 512
        nchunks = (GCP + CH - 1) // CH
        for c in range(nchunks):
            w = min(CH, GCP - c * CH)
            sl = slice(c * CH, c * CH + w)
            psb = pp.tile([D, CH], f32, tag="mid")
            nc.tensor.matmul(psb[:, :w], lhsT=ones1[:], rhs=inv[:, sl],
                             start=True, stop=True)
            nc.vector.tensor_tensor(out=meanT[:, sl], in0=acc1[0:D, sl],
                                    in1=psb[:, :w], op=ALU.mult)
        for c in range(nchunks):
            w = min(CH, GCP - c * CH)
            sl = slice(c * CH, c * CH + w)
            psg = pp.tile([D, CH], f32, tag="mid")
            nc.tensor.matmul(psg[:, :w], lhsT=w_sb[:], rhs=meanT[:, sl],
                             start=True, stop=True)
            nc.scalar.activation(tgT[0:D, 0, sl], psg[:, :w], AF.Tanh)

        nc.sync.dma_start(tgscratch[:], tgT[0:D, 0, :])
        nc.sync.dma_start(tgT[D:P, 1, :], tgscratch[:])

        # ---------------- pass 2: coefs + weighted seg-sum ------------------
        for t in range(n_meg):
            xs_t = px.tile([P, T, DP2], bf16, tag="xs")
            nc.sync.dma_start(xs_t[:], xs[t])
            xts_t = pxt.tile([P, T // 2, P], bf16, tag="xts")
            nc.sync.dma_start(xts_t[:], xts[t])
            c0a = c0_of(t, engines=[ENG.Activation])
            tgwin = pk.tile([P, 2, WIN], bf16, tag="tgwin")
            nc.scalar.copy(tgwin[:], tgT[:, :, bass.ds(c0a, WIN)])

            m = build_M(xs_t[:, :, D + 1])
            psD = ppd.tile([P, T, WIN], f32, tag="psD")
            for jj in range(T // 2):
                nc.tensor.matmul(psD[:, 2 * jj:2 * jj + 2, :],
                                 lhsT=xts_t[:, jj, :],
                                 rhs=tgwin[:, :, :],
                                 start=True, stop=True)
            dsb = pm.tile([P, T, WIN], bf16, tag="dsb")
            nc.scalar.copy(dsb[:], psD[:])
            a_pick = pm.tile([P, T, WIN], bf16, tag="apick")
            nc.vector.tensor_tensor(out=a_pick[:], in0=m[:], in1=dsb[:],
                                    op=ALU.mult)
            s = pk.tile([P, T], f32, tag="s")
            nc.vector.tensor_reduce(
                out=s[:], in_=a_pick[:], axis=mybir.AxisListType.X,
                op=ALU.add)
            coef = pk.tile([P, T], bf16, tag="coef")
            nc.scalar.activation(coef[:], s[:], AF.Sigmoid)
            cexp = pm.tile([P, T, WIN], bf16, tag="cexp")
            nc.scalar.copy(cexp[:], coef[:].to_broadcast([P, T, WIN]))
            mp = pm.tile([P, T, WIN], bf16, tag="Mp")
            nc.vector.tensor_tensor(out=mp[:], in0=m[:], in1=cexp[:],
                                    op=ALU.mult)
            ps3 = pp1.tile([DP1, WIN], f32, tag="pacc")
            for j in range(T):
                nc.tensor.matmul(ps3[:], lhsT=xs_t[:, j, 0:DP1], rhs=mp[:, j, :],
                                 start=(j == 0), stop=(j == T - 1))
            c0v = c0_of(t, engines=[ENG.DVE])
            a = acc2[:, bass.ds(c0v, WIN)]
            nc.vector.tensor_tensor(out=a, in0=a, in1=ps3[:], op=ALU.add)

        # ---------------- end: transpose acc2 -> out ------------------------
        for c in range(NCHUNK):
            pst = pp.tile([P, D], f32, tag="mid")
            nc.tensor.transpose(pst[:], acc2[0:D, c * P:(c + 1) * P],
                                ident_sb[0:D, 0:D])
            oc = pk.tile([P, D], f32, tag="oc")
            nc.scalar.copy(oc[:], pst[:])
            nc.sync.dma_start(out[c * P:(c + 1) * P, :], oc[:])

    nc.compile()
    return nc


# ----------------------------------------------------------------------------
# entry point
# ----------------------------------------------------------------------------

_CACHE = {}


def kernel(x, batch, size, W):
    global LAST_EXEC_NS
    from concourse import bass_utils

    x = np.asarray(x, dtype=np.float32)
    batch_np = np.asarray(batch).astype(np.int64)
    W = np.asarray(W, dtype=np.float32)
    n = x.shape[0]
    size = int(size)
    cores = CORES

    gsplit, nsplit = _shard_plan(batch_np, size, cores)
    max_nodes = max(nsplit[k + 1] - nsplit[k] for k in range(cores))
    n_meg = max(1, -(-max_nodes // NT))

    iota, ident = _host_consts()
    in_maps = []
    gcs = []
    for k in range(cores):
        m, gc = _prep_core(x, batch_np, gsplit[k], gsplit[k + 1],
                           nsplit[k], nsplit[k + 1], n_meg)
        m["wmat"] = W
        m["iotac"] = iota
        m["identc"] = ident
        in_maps.append(m)
        gcs.append(gc)

    if n_meg not in _CACHE:
        _CACHE[n_meg] = build_nc(n_meg)
    nc = _CACHE[n_meg]

    trace = os.environ.get("BASS_KERNEL_TRACE", "0") == "1"
    res = bass_utils.run_bass_kernel_spmd(nc, in_maps,
                                          core_ids=list(range(cores)),
                                          trace=trace)
    LAST_EXEC_NS = res.exec_time_ns
    outs = [res.results[k]["out"][:gcs[k]] for k in range(cores)]
    full = np.concatenate(outs, axis=0)
    if full.shape[0] < size:
        full = np.concatenate(
            [full, np.zeros((size - full.shape[0], D), np.float32)], axis=0)
    return np.ascontiguousarray(full[:size], dtype=np.float32)
